# revision 18
# baseline (speedup 1.0000x reference)
"""Distributed AND-convolution (Dempster combination / FWHT-style) for 8 TRN2 cores.

out = mobius(zeta(m1) * zeta(m2)) over 24 bit-axes, L = 2^24.

Sharding: top 3 bits (h = k[23:21]) = core id. Per core per channel: 2^21
elements, SBUF (128, 16384): partition p = l[20:14], free f = l[13:0].

v2 pipeline (overlap-first):
 - slabs = 4 contiguous 4096-col blocks; fwd order 3,2,0,1-load with A2As
   emitted 3,2,1,0 so the first collective fires ~55us in (vs ~190 in v1).
 - cross-slab zeta stages (free bits 13,12) are woven in as whole-slab
   adds at the right points (S2+=S3, S1+=S3, S0+=S2orig, S0+=S1cur).
 - ALL inverse work except the pc-bit (k[20:18]) mobius runs BEFORE the
   back A2A (free-bit stages / c+pm mobius commute with pc mobius), so
   the post-back-A2A tail is just kron(M3,I16) matmul + output DMA.
 - back A2As merged into 2x4MiB (slabs {3,2} and {1,0}).
 - collective train order: F3 F2 F1 B32 F0 B10.
"""
import sys
sys.path.insert(0, '/opt/trn_rl_repo')
import numpy as np

NCORES = 8
P = 128
F = 16384
BLK = 512            # matmul block
NBLK = F // BLK      # 32
NSLAB = 4
MSL = NBLK // NSLAB  # 8 blocks per slab
SLAB = F // NSLAB    # 4096 contiguous cols


def _zeta_mat(nbits):
    idx = np.arange(1 << nbits)
    return ((idx[:, None] & idx[None, :]) == idx[None, :]).astype(np.float32)


def _mobius_mat(nbits):
    idx = np.arange(1 << nbits)
    sup = (idx[:, None] & idx[None, :]) == idx[None, :]
    pc = np.array([bin(x).count("1") for x in range(1 << nbits)])
    signs = (-1.0) ** pc[idx[:, None] & ~idx[None, :]]
    return (sup * signs).astype(np.float32)


def build_kernel():
    import concourse.bacc as bacc
    import concourse.tile as tile
    from concourse import mybir

    f32 = mybir.dt.float32
    nc = bacc.Bacc("TRN2", target_bir_lowering=False, debug=False, num_devices=NCORES)

    m1_in = nc.dram_tensor("m1", [P, F], f32, kind="ExternalInput")
    m2_in = nc.dram_tensor("m2", [P, F], f32, kind="ExternalInput")
    out_t = nc.dram_tensor("out", [P, F], f32, kind="ExternalOutput")

    WZ7_d = nc.inline_tensor(_zeta_mat(7), name="WZ7")
    WZ3_d = nc.inline_tensor(np.kron(_zeta_mat(3), np.eye(16, dtype=np.float32)), name="WZ3x")
    # c-bit + pm-bit mobius (pre back-A2A, partitions = (c, pm))
    WM34_d = nc.inline_tensor(np.kron(_mobius_mat(3), _mobius_mat(4)), name="WM34")
    # flo-bit (free bits 6:0) mobius for the Fb transpose pass
    WM7F_d = nc.inline_tensor(_mobius_mat(7), name="WM7F")
    # pc-bit mobius (post back-A2A, partitions = (pc, pm))
    WM3I_d = nc.inline_tensor(np.kron(_mobius_mat(3), np.eye(16, dtype=np.float32)), name="WM3I")

    with tile.TileContext(nc) as tc:
        with tc.tile_pool(name="sbuf", bufs=1) as pool, \
             tc.tile_pool(name="chunks", bufs=4) as cpool, \
             tc.tile_pool(name="psum", bufs=2, space="PSUM") as psum, \
             tc.tile_pool(name="dram", bufs=1, space="DRAM") as dram:

            wz7 = pool.tile([P, P], f32)
            wz3 = pool.tile([P, P], f32)
            wm34 = pool.tile([P, P], f32)
            wm7f = pool.tile([P, P], f32)
            wm3i = pool.tile([P, P], f32)
            nc.sync.dma_start(out=wz7[:], in_=WZ7_d[:])
            nc.sync.dma_start(out=wz3[:], in_=WZ3_d[:])
            nc.sync.dma_start(out=wm34[:], in_=WM34_d[:])
            nc.sync.dma_start(out=wm7f[:], in_=WM7F_d[:])
            nc.sync.dma_start(out=wm3i[:], in_=WM3I_d[:])

            A = pool.tile([P, F], f32)
            B = pool.tile([P, F], f32)
            TA = pool.tile([P, SLAB], f32)   # fwd F-pair transpose scratch
            TB = pool.tile([P, SLAB], f32)

            cc_in = [dram.tile([NCORES, 2, 16, SLAB], f32, tag=f"cci{s}", name=f"cci{s}") for s in range(NSLAB)]
            cc_out = [dram.tile([NCORES, 2, 16, SLAB], f32, tag=f"cco{s}", name=f"cco{s}") for s in range(NSLAB)]
            # back A2As: one per slab (1 channel, 2 MiB each)
            cc2_in = [dram.tile([NCORES, 16, SLAB], f32, tag=f"c2i{g}", name=f"c2i{g}") for g in range(4)]
            cc2_out = [dram.tile([NCORES, 16, SLAB], f32, tag=f"c2o{g}", name=f"c2o{g}") for g in range(4)]

            def cols(s):
                return (s * SLAB, (s + 1) * SLAB)

            def tt(alu, lo, hi):
                f = getattr(nc.vector, "tensor_add" if alu == "add" else "tensor_sub")
                f(lo, lo, hi)

            def slab_add(t, sd, ss, alu="add"):
                # two half-ops so consecutive adds on one slab form two
                # shorter dependency chains (hides DVE sem latency)
                c0, _ = cols(sd)
                d0, _ = cols(ss)
                for h in (0, 1):
                    tt(alu, t[:, c0 + h * 2048:c0 + (h + 1) * 2048],
                       t[:, d0 + h * 2048:d0 + (h + 1) * 2048])

            def slab_stages(t, s, alu):
                # intra-slab free-bit stages j=0..11 on contiguous slab s
                c0, c1 = cols(s)
                sl = t[:, c0:c1]
                for j in range(12):
                    w = sl.rearrange("p (a two b) -> p a two b", two=2, b=1 << j)
                    tt(alu, w[:, :, 0, :], w[:, :, 1, :])

            def fwd_stages_hi(s):
                # free-bit stages j=7..11 for BOTH channels (flo 6:0 and the
                # partition bits are handled by the fwd F-pair on TensorE).
                # Chains interleaved (channel x half-slab) to hide DVE sem
                # latency; j=11 crosses the halves once.
                c0, c1 = cols(s)
                for j in range(7, 11):
                    for t in (A, B):
                        for h in (0, 1):
                            sl = t[:, c0 + h * 2048:c0 + (h + 1) * 2048]
                            w = sl.rearrange("p (a two b) -> p a two b", two=2, b=1 << j)
                            tt("add", w[:, :, 0, :], w[:, :, 1, :])
                for t in (A, B):
                    sl = t[:, c0:c1]
                    w = sl.rearrange("p (a two b) -> p a two b", two=2, b=1 << 11)
                    tt("add", w[:, :, 0, :], w[:, :, 1, :])

            def mm_block(t, w, blk):
                # t[:, blk] = w.T @ t[:, blk] ; evac via ACT
                ps = psum.tile([P, BLK], f32, tag="ps_mm")
                sl = t[:, blk * BLK:(blk + 1) * BLK]
                nc.tensor.matmul(ps[:], lhsT=w[:], rhs=sl, start=True, stop=True)
                nc.scalar.copy(sl, ps[:])

            def load_slab(t, src, s):
                # scalar-engine descgen: keeps input loads OFF the sync queue
                # (which carries staging DMAs that block on compute)
                c0, c1 = cols(s)
                nc.scalar.dma_start(out=t[:, c0:c1], in_=src[:, c0:c1])

            def fwd_local(s):
                # per-slab local fwd work for both channels. The partition-bit
                # zeta (WZ7) and the flo (free 6:0) zeta ride on TensorE as a
                # fused transpose-matmul pair (Fa: contract partitions with
                # WZ7^T while transposing each 128-chunk into scratch; Fb:
                # contract flo with WZ7^T, transposing back). DVE only runs
                # free bits 11:7, so the per-slab DVE chain is short and the
                # A2A cadence is TensorE/A2A-limited.
                c0, c1 = cols(s)
                for t, scr in ((A, TA), (B, TB)):
                    for m in range(MSL):
                        blk = s * MSL + m
                        psF = psum.tile([P, BLK], f32, tag="psA")
                        for j in range(4):
                            ch = blk * BLK + j * P
                            nc.tensor.matmul(psF[:, j * P:(j + 1) * P],
                                             lhsT=t[:, ch:ch + P], rhs=wz7[:],
                                             start=True, stop=True)
                        nc.scalar.copy(scr[:, m * BLK:(m + 1) * BLK], psF[:])
                    for m in range(MSL):
                        psF = psum.tile([P, BLK], f32, tag="psB")
                        for j in range(4):
                            ch = m * BLK + j * P
                            nc.tensor.matmul(psF[:, j * P:(j + 1) * P],
                                             lhsT=scr[:, ch:ch + P], rhs=wz7[:],
                                             start=True, stop=True)
                        nc.scalar.copy(t[:, c0 + m * BLK:c0 + (m + 1) * BLK], psF[:])
                fwd_stages_hi(s)

            def stage_fwd(s):
                c0, c1 = cols(s)
                for d in range(NCORES):
                    nc.sync.dma_start(out=cc_in[s][d, 0], in_=A[16 * d:16 * (d + 1), c0:c1])
                    nc.sync.dma_start(out=cc_in[s][d, 1], in_=B[16 * d:16 * (d + 1), c0:c1])
                nc.gpsimd.collective_compute(
                    "AllToAll", mybir.AluOpType.bypass,
                    replica_groups=[list(range(NCORES))],
                    ins=[cc_in[s][:].opt()], outs=[cc_out[s][:].opt()],
                )

            def mid_slab(s):
                # recv fwd A2A, h-conv: zeta3 on both channels, product into A.
                # Then the inverse transform via two fused transpose-matmul
                # passes on TensorE: Fa applies mobius(c,pm) (partition dim)
                # while transposing each 128-chunk into B; Fb applies
                # mobius(flo = free bits 6:0) while transposing back into A.
                # Remaining free bits 11:7 via 5 DVE stages. (Bits 13:12 are
                # the cross-slab weave; pc bits get WM3I post back-A2A.)
                c0, c1 = cols(s)
                for c in range(NCORES):
                    nc.gpsimd.dma_start(out=A[16 * c:16 * (c + 1), c0:c1], in_=cc_out[s][c, 0])
                    nc.gpsimd.dma_start(out=B[16 * c:16 * (c + 1), c0:c1], in_=cc_out[s][c, 1])
                for m in range(MSL):
                    blk = s * MSL + m
                    sa = A[:, blk * BLK:(blk + 1) * BLK]
                    sb = B[:, blk * BLK:(blk + 1) * BLK]
                    psA = psum.tile([P, BLK], f32, tag="psA")
                    psB = psum.tile([P, BLK], f32, tag="psB")
                    nc.tensor.matmul(psA[:], lhsT=wz3[:], rhs=sa, start=True, stop=True)
                    nc.tensor.matmul(psB[:], lhsT=wz3[:], rhs=sb, start=True, stop=True)
                    qa = cpool.tile([P, BLK], f32, tag="qa")
                    nc.scalar.copy(qa[:], psA[:])
                    nc.vector.tensor_mul(sa, qa[:], psB[:])
                # Fa: A chunks -> (mobius(c,pm))^T-transformed transposed chunks in B
                for m in range(MSL):
                    blk = s * MSL + m
                    psFa = psum.tile([P, BLK], f32, tag="psA")
                    for j in range(4):
                        ch = blk * BLK + j * P
                        nc.tensor.matmul(psFa[:, j * P:(j + 1) * P],
                                         lhsT=A[:, ch:ch + P], rhs=wm34[:],
                                         start=True, stop=True)
                    nc.scalar.copy(B[:, blk * BLK:(blk + 1) * BLK], psFa[:])
                # Fb: B chunks -> flo-mobius + transpose back into A
                for m in range(MSL):
                    blk = s * MSL + m
                    psFb = psum.tile([P, BLK], f32, tag="psB")
                    for j in range(4):
                        ch = blk * BLK + j * P
                        nc.tensor.matmul(psFb[:, j * P:(j + 1) * P],
                                         lhsT=B[:, ch:ch + P], rhs=wm7f[:],
                                         start=True, stop=True)
                    nc.scalar.copy(A[:, blk * BLK:(blk + 1) * BLK], psFb[:])
                # inverse free-bit stages j=7..11 (flo 6:0 done by Fb),
                # halves interleaved to hide DVE sem latency
                for j in range(7, 11):
                    for h in (0, 1):
                        sl = A[:, c0 + h * 2048:c0 + (h + 1) * 2048]
                        w = sl.rearrange("p (a two b) -> p a two b", two=2, b=1 << j)
                        tt("sub", w[:, :, 0, :], w[:, :, 1, :])
                sl = A[:, c0:c1]
                w = sl.rearrange("p (a two b) -> p a two b", two=2, b=1 << 11)
                tt("sub", w[:, :, 0, :], w[:, :, 1, :])

            def stage_back(g, slabs):
                # stage two inv'd slabs + back A2A
                for d in range(NCORES):
                    for i, s in enumerate(slabs):
                        c0, c1 = cols(s)
                        nc.sync.dma_start(out=cc2_in[g][d, :, i * SLAB:(i + 1) * SLAB],
                                          in_=A[16 * d:16 * (d + 1), c0:c1])
                nc.gpsimd.collective_compute(
                    "AllToAll", mybir.AluOpType.bypass,
                    replica_groups=[list(range(NCORES))],
                    ins=[cc2_in[g][:].opt()], outs=[cc2_out[g][:].opt()],
                )

            def tail_slabs(g, slabs):
                # recv back A2A into B, pc-mobius matmul, stream out
                for i, s in enumerate(slabs):
                    c0, c1 = cols(s)
                    for d in range(NCORES):
                        nc.gpsimd.dma_start(out=B[16 * d:16 * (d + 1), c0:c1],
                                            in_=cc2_out[g][d, :, i * SLAB:(i + 1) * SLAB])
                    for m in range(MSL):
                        mm_block(B, wm3i, s * MSL + m)
                    nc.sync.dma_start(out=out_t[:, c0:c1], in_=B[:, c0:c1])

            # ---------------- forward pipeline ----------------
            # ALL forward work is emitted before any mid work: engine queues
            # execute in order, so a mid op waiting on an A2A result must not
            # sit ahead of independent fwd work in the DVE/ACT/TE queues.
            # all input loads issued upfront: descgens fire immediately, DMA
            # hardware streams the 16 MiB in the background
            for s in (3, 2, 1, 0):
                for t, src in ((A, m1_in), (B, m2_in)):
                    load_slab(t, src, s)

            fwd_local(3)
            stage_fwd(3)                       # F3

            fwd_local(2)
            for t in (A, B):
                slab_add(t, 2, 3)              # S2 += S3
            stage_fwd(2)                       # F2

            fwd_local(1)
            for t in (A, B):
                slab_add(t, 1, 3)              # S1 += S3
            stage_fwd(1)                       # F1

            fwd_local(0)
            for t in (A, B):
                slab_add(t, 0, 1)              # S0 += (S1+S3)
                slab_add(t, 0, 2)              # S0 += (S2+S3)
                slab_add(t, 0, 3, "sub")       # S0 -= S3  => S0+S1+S2+S3
            stage_fwd(0)                       # F0

            mid_slab(3)
            stage_back(0, (3,))                # B3
            mid_slab(2)
            slab_add(A, 2, 3, "sub")           # S2 -= S3
            stage_back(1, (2,))                # B2
            tail_slabs(0, (3,))
            mid_slab(1)
            slab_add(A, 1, 3, "sub")           # S1 -= S3
            stage_back(2, (1,))                # B1
            tail_slabs(1, (2,))
            mid_slab(0)
            slab_add(A, 0, 1, "sub")
            slab_add(A, 0, 2, "sub")
            slab_add(A, 0, 3, "sub")
            stage_back(3, (0,))                # B0
            tail_slabs(2, (1,))
            tail_slabs(3, (0,))

    nc.compile()
    return nc


_NC_CACHE = None


def kernel(m12: np.ndarray) -> np.ndarray:
    global _NC_CACHE
    from concourse.bass_utils import run_bass_kernel_spmd

    if _NC_CACHE is None:
        _NC_CACHE = build_kernel()
    nc = _NC_CACHE

    m12 = np.ascontiguousarray(np.asarray(m12, dtype=np.float32))
    Bsz, C, L = m12.shape
    S = L // NCORES
    in_maps = []
    for c in range(NCORES):
        in_maps.append({
            "m1": m12[0, 0, c * S:(c + 1) * S].reshape(P, F),
            "m2": m12[0, 1, c * S:(c + 1) * S].reshape(P, F),
        })
    try:
        res = run_bass_kernel_spmd(nc, in_maps, core_ids=list(range(NCORES)))
    except Exception:
        # transient NRT/device hiccups have been observed; retry once
        import time
        time.sleep(5)
        res = run_bass_kernel_spmd(nc, in_maps, core_ids=list(range(NCORES)))
    out = np.concatenate([res.results[c]["out"].reshape(-1) for c in range(NCORES)])
    return out.reshape(1, L, 1, 1)


if __name__ == "__main__":
    m12 = np.load("/root/problem/m12.npy")
    out = kernel(m12)
    exp = np.load("/root/problem/expected.npy")
    err = np.abs(out - exp).max()
    scale = np.abs(exp).max()
    print(f"absmax err {err:.4g} scale {scale:.4g} rel {err/scale:.3e}")


# revision 22
# speedup vs baseline: 1.0671x; 1.0671x over previous
"""Distributed AND-convolution (Dempster combination / FWHT-style) for 8 TRN2 cores.

out = mobius(zeta(m1) * zeta(m2)) over 24 bit-axes, L = 2^24.

Sharding: top 3 bits (h = k[23:21]) = core id. Per core per channel: 2^21
elements, SBUF (128, 16384): partition p = l[20:14], free f = l[13:0].

Slab pipelining: f-blocks of 512 are interleaved into 4 slabs
(slab s = blocks with blk%4 == s, blk = f>>9). Free-bit butterfly stages
j in {0..8, 11..13} are intra-slab; j in {9, 10} are cross-slab and run
full-tile at the start (fwd) / end (inv). Each slab flows independently:
  fwd stages -> zeta matmul (7 partition bits) -> AllToAll ->
  h-conv (zeta3/product/mobius3 matmuls on (c,pm) partitions) ->
  AllToAll back -> mobius matmul -> inv stages
so DVE/GPSIMD butterflies, TensorE matmuls and the collectives overlap.
"""
import sys
sys.path.insert(0, '/opt/trn_rl_repo')
import numpy as np

NCORES = 8
P = 128
F = 16384
BLK = 512            # matmul / block granularity
NBLK = F // BLK      # 32
NSLAB = 4
MSL = NBLK // NSLAB  # 8 blocks per slab
SLAB = F // NSLAB    # 4096 contiguous


def _zeta_mat(nbits):
    idx = np.arange(1 << nbits)
    return ((idx[:, None] & idx[None, :]) == idx[None, :]).astype(np.float32)


def _mobius_mat(nbits):
    idx = np.arange(1 << nbits)
    sup = (idx[:, None] & idx[None, :]) == idx[None, :]
    pc = np.array([bin(x).count("1") for x in range(1 << nbits)])
    signs = (-1.0) ** pc[idx[:, None] & ~idx[None, :]]
    return (sup * signs).astype(np.float32)


def build_kernel():
    import concourse.bacc as bacc
    import concourse.tile as tile
    from concourse import mybir

    f32 = mybir.dt.float32
    nc = bacc.Bacc("TRN2", target_bir_lowering=False, debug=False, num_devices=NCORES)

    m1_in = nc.dram_tensor("m1", [P, F], f32, kind="ExternalInput")
    m2_in = nc.dram_tensor("m2", [P, F], f32, kind="ExternalInput")
    out_t = nc.dram_tensor("out", [P, F], f32, kind="ExternalOutput")

    WZ7_d = nc.inline_tensor(_zeta_mat(7), name="WZ7")
    WM7_d = nc.inline_tensor(_mobius_mat(7), name="WM7")
    WZ3_d = nc.inline_tensor(np.kron(_zeta_mat(3), np.eye(16, dtype=np.float32)), name="WZ3x")
    WM3_d = nc.inline_tensor(np.kron(_mobius_mat(3), np.eye(16, dtype=np.float32)), name="WM3x")

    with tile.TileContext(nc) as tc:
        with tc.tile_pool(name="sbuf", bufs=1) as pool, \
             tc.tile_pool(name="chunks", bufs=4) as cpool, \
             tc.tile_pool(name="psum", bufs=2, space="PSUM") as psum, \
             tc.tile_pool(name="dram", bufs=1, space="DRAM") as dram:

            wz7 = pool.tile([P, P], f32)
            wm7 = pool.tile([P, P], f32)
            wz3 = pool.tile([P, P], f32)
            wm3 = pool.tile([P, P], f32)
            nc.sync.dma_start(out=wz7[:], in_=WZ7_d[:])
            nc.sync.dma_start(out=wm7[:], in_=WM7_d[:])
            nc.sync.dma_start(out=wz3[:], in_=WZ3_d[:])
            nc.sync.dma_start(out=wm3[:], in_=WM3_d[:])

            A = pool.tile([P, F], f32)
            B = pool.tile([P, F], f32)
            nc.sync.dma_start(out=A[:], in_=m1_in[:])
            nc.scalar.dma_start(out=B[:], in_=m2_in[:])

            cc_in = [dram.tile([NCORES, 2, 16, MSL * BLK], f32, tag=f"cci{s}", name=f"cci{s}") for s in range(NSLAB)]
            cc_out = [dram.tile([NCORES, 2, 16, MSL * BLK], f32, tag=f"cco{s}", name=f"cco{s}") for s in range(NSLAB)]
            cc2_in = [dram.tile([NCORES, 16, MSL * BLK], f32, tag=f"c2i{s}", name=f"c2i{s}") for s in range(NSLAB)]
            cc2_out = [dram.tile([NCORES, 16, MSL * BLK], f32, tag=f"c2o{s}", name=f"c2o{s}") for s in range(NSLAB)]

            ADD = "add"

            def tt(eng, alu, lo, hi):
                f = getattr(nc.vector if eng == "v" else nc.gpsimd,
                            "tensor_add" if alu == ADD else "tensor_sub")
                f(lo, lo, hi)

            def bf_split(alu, lo, hi):
                # DVE only: GPSIMD shares SBUF ports with DVE, so
                # concurrent fp32 TT on both engines halves each rate.
                tt("v", alu, lo, hi)

            def s13(t, alu):
                # bit-13 butterfly: cols [0:8k] (+/-)= cols [8k:16k]
                tt("v", alu, t[:, 0:8192], t[:, 8192:16384])

            def s12a(t, alu):
                tt("v", alu, t[:, 0:4096], t[:, 4096:8192])

            def s12b(t, alu):
                tt("v", alu, t[:, 8192:12288], t[:, 12288:16384])

            def slab_stages(t, s, alu):
                # intra-slab stages j=0..11 on contiguous slab s, split into
                # 2048-col half-chains (j=0..10 act within a half; j=11
                # crosses once). Each DVE op carries ~1.7us of semaphore
                # latency on its predecessor, so splitting one 12-deep chain
                # into two interleaved 11-deep chains hides half of it.
                c0 = s * SLAB
                for j in range(11):
                    for h in (0, 1):
                        sl = t[:, c0 + h * 2048:c0 + (h + 1) * 2048]
                        w = sl.rearrange("p (a two b) -> p a two b", two=2, b=1 << j)
                        bf_split(alu, w[:, :, 0, :], w[:, :, 1, :])
                sl = t[:, c0:c0 + SLAB]
                w = sl.rearrange("p (a two b) -> p a two b", two=2, b=1 << 11)
                bf_split(alu, w[:, :, 0, :], w[:, :, 1, :])

            def slab_stages_pair(s, alu):
                # both channels, chains interleaved 4-ways (channel x half)
                c0 = s * SLAB
                for j in range(11):
                    for t in (A, B):
                        for h in (0, 1):
                            sl = t[:, c0 + h * 2048:c0 + (h + 1) * 2048]
                            w = sl.rearrange("p (a two b) -> p a two b", two=2, b=1 << j)
                            bf_split(alu, w[:, :, 0, :], w[:, :, 1, :])
                for t in (A, B):
                    sl = t[:, c0:c0 + SLAB]
                    w = sl.rearrange("p (a two b) -> p a two b", two=2, b=1 << 11)
                    bf_split(alu, w[:, :, 0, :], w[:, :, 1, :])

            def mm_block(t, w, blk):
                ps = psum.tile([P, BLK], f32, tag="ps_mm")
                sl = t[:, blk * BLK:(blk + 1) * BLK]
                nc.tensor.matmul(ps[:], lhsT=w[:], rhs=sl, start=True, stop=True)
                nc.scalar.copy(sl, ps[:])

            def slab_ap(t, s, prange=None):
                if prange is None:
                    return t[:, s * SLAB:(s + 1) * SLAB]
                return t[prange[0]:prange[1], s * SLAB:(s + 1) * SLAB]

            def emit_phase3_back(s):
                # recv, h-conv, back-send, back A2A for slab s
                for c in range(NCORES):
                    nc.gpsimd.dma_start(out=slab_ap(A, s, (16 * c, 16 * (c + 1))), in_=cc_out[s][c, 0])
                    nc.gpsimd.dma_start(out=slab_ap(B, s, (16 * c, 16 * (c + 1))), in_=cc_out[s][c, 1])
                for m in range(MSL):
                    blk = s * MSL + m
                    sa = A[:, blk * BLK:(blk + 1) * BLK]
                    sb = B[:, blk * BLK:(blk + 1) * BLK]
                    psA = psum.tile([P, BLK], f32, tag="psA")
                    psB = psum.tile([P, BLK], f32, tag="psB")
                    nc.tensor.matmul(psA[:], lhsT=wz3[:], rhs=sa, start=True, stop=True)
                    nc.tensor.matmul(psB[:], lhsT=wz3[:], rhs=sb, start=True, stop=True)
                    qa = cpool.tile([P, BLK], f32, tag="qa")
                    nc.scalar.copy(qa[:], psA[:])
                    pr = cpool.tile([P, BLK], f32, tag="pr")
                    nc.vector.tensor_mul(pr[:], qa[:], psB[:])
                    psU = psum.tile([P, BLK], f32, tag="psU")
                    nc.tensor.matmul(psU[:], lhsT=wm3[:], rhs=pr[:], start=True, stop=True)
                    nc.scalar.copy(sa, psU[:])
                for c in range(NCORES):
                    nc.sync.dma_start(out=cc2_in[s][c], in_=slab_ap(A, s, (16 * c, 16 * (c + 1))))
                nc.gpsimd.collective_compute(
                    "AllToAll", mybir.AluOpType.bypass,
                    replica_groups=[list(range(NCORES))],
                    ins=[cc2_in[s][:].opt()], outs=[cc2_out[s][:].opt()],
                )

            # ---------- phase 1 + forward A2As, back path interleaved 2 behind ----------
            # s13/s12a as interleaved half-ops (independent chains hide the
            # per-op DVE semaphore latency)
            for h in (0, 1):
                for t in (A, B):
                    tt("v", ADD, t[:, h * 4096:(h + 1) * 4096],
                       t[:, 8192 + h * 4096:8192 + (h + 1) * 4096])
            for h in (0, 1):
                for t in (A, B):
                    tt("v", ADD, t[:, h * 2048:(h + 1) * 2048],
                       t[:, 4096 + h * 2048:4096 + (h + 1) * 2048])
            for s in range(NSLAB):
                if s == 2:
                    # s12b (writes [8k:12k], feeds slabs 2,3) deferred out of
                    # the slab-0 critical path
                    for h in (0, 1):
                        for t in (A, B):
                            tt("v", ADD, t[:, 8192 + h * 2048:8192 + (h + 1) * 2048],
                               t[:, 12288 + h * 2048:12288 + (h + 1) * 2048])
                slab_stages_pair(s, ADD)
                for t in (A, B):
                    for m in range(MSL):
                        mm_block(t, wz7, s * MSL + m)
                for d in range(NCORES):
                    nc.sync.dma_start(out=cc_in[s][d, 0], in_=slab_ap(A, s, (16 * d, 16 * (d + 1))))
                    nc.sync.dma_start(out=cc_in[s][d, 1], in_=slab_ap(B, s, (16 * d, 16 * (d + 1))))
                nc.gpsimd.collective_compute(
                    "AllToAll", mybir.AluOpType.bypass,
                    replica_groups=[list(range(NCORES))],
                    ins=[cc_in[s][:].opt()], outs=[cc_out[s][:].opt()],
                )
                if s >= 2:
                    emit_phase3_back(s - 2)
            for s in (NSLAB - 2, NSLAB - 1):
                emit_phase3_back(s)

            # ---------- phase 5 per slab ----------
            for s in range(NSLAB):
                for d in range(NCORES):
                    nc.gpsimd.dma_start(out=slab_ap(B, s, (16 * d, 16 * (d + 1))), in_=cc2_out[s][d])
                for m in range(MSL):
                    mm_block(B, wm7, s * MSL + m)
                slab_stages(B, s, "sub")
                if s == 1:
                    # needs only slabs 0,1 — run while slabs 2,3 are in flight
                    for h in (0, 1):
                        tt("v", "sub", B[:, h * 2048:(h + 1) * 2048],
                           B[:, 4096 + h * 2048:4096 + (h + 1) * 2048])

            # ---------- inverse cross-slab stages + streamed output ----------
            for h in (0, 1):
                tt("v", "sub", B[:, 8192 + h * 2048:8192 + (h + 1) * 2048],
                   B[:, 12288 + h * 2048:12288 + (h + 1) * 2048])
            nc.sync.dma_start(out=out_t[:, 12288:16384], in_=B[:, 12288:16384])
            for h in (0, 1):
                tt("v", "sub", B[:, h * 4096:(h + 1) * 4096],
                   B[:, 8192 + h * 4096:8192 + (h + 1) * 4096])
            nc.sync.dma_start(out=out_t[:, 8192:12288], in_=B[:, 8192:12288])
            nc.sync.dma_start(out=out_t[:, 0:8192], in_=B[:, 0:8192])

    nc.compile()
    return nc


_NC_CACHE = None


def kernel(m12: np.ndarray) -> np.ndarray:
    global _NC_CACHE
    from concourse.bass_utils import run_bass_kernel_spmd

    if _NC_CACHE is None:
        _NC_CACHE = build_kernel()
    nc = _NC_CACHE

    m12 = np.ascontiguousarray(np.asarray(m12, dtype=np.float32))
    Bsz, C, L = m12.shape
    S = L // NCORES
    in_maps = []
    for c in range(NCORES):
        in_maps.append({
            "m1": m12[0, 0, c * S:(c + 1) * S].reshape(P, F),
            "m2": m12[0, 1, c * S:(c + 1) * S].reshape(P, F),
        })
    try:
        res = run_bass_kernel_spmd(nc, in_maps, core_ids=list(range(NCORES)))
    except Exception:
        # transient NRT/device hiccups have been observed; retry once
        import time
        time.sleep(5)
        res = run_bass_kernel_spmd(nc, in_maps, core_ids=list(range(NCORES)))
    out = np.concatenate([res.results[c]["out"].reshape(-1) for c in range(NCORES)])
    return out.reshape(1, L, 1, 1)


if __name__ == "__main__":
    m12 = np.load("/root/problem/m12.npy")
    out = kernel(m12)
    exp = np.load("/root/problem/expected.npy")
    err = np.abs(out - exp).max()
    scale = np.abs(exp).max()
    print(f"absmax err {err:.4g} scale {scale:.4g} rel {err/scale:.3e}")



# revision 23
# speedup vs baseline: 1.0855x; 1.0173x over previous
"""Distributed AND-convolution (Dempster combination / FWHT-style) for 8 TRN2 cores.

out = mobius(zeta(m1) * zeta(m2)) over 24 bit-axes, L = 2^24.

Sharding: top 3 bits (h = k[23:21]) = core id. Per core per channel: 2^21
elements, SBUF (128, 16384): partition p = l[20:14], free f = l[13:0].

Slab pipelining: f-blocks of 512 are interleaved into 4 slabs
(slab s = blocks with blk%4 == s, blk = f>>9). Free-bit butterfly stages
j in {0..8, 11..13} are intra-slab; j in {9, 10} are cross-slab and run
full-tile at the start (fwd) / end (inv). Each slab flows independently:
  fwd stages -> zeta matmul (7 partition bits) -> AllToAll ->
  h-conv (zeta3/product/mobius3 matmuls on (c,pm) partitions) ->
  AllToAll back -> mobius matmul -> inv stages
so DVE/GPSIMD butterflies, TensorE matmuls and the collectives overlap.
"""
import sys
sys.path.insert(0, '/opt/trn_rl_repo')
import numpy as np

NCORES = 8
P = 128
F = 16384
BLK = 512            # matmul / block granularity
NBLK = F // BLK      # 32
NSLAB = 4
MSL = NBLK // NSLAB  # 8 blocks per slab
SLAB = F // NSLAB    # 4096 contiguous


def _zeta_mat(nbits):
    idx = np.arange(1 << nbits)
    return ((idx[:, None] & idx[None, :]) == idx[None, :]).astype(np.float32)


def _mobius_mat(nbits):
    idx = np.arange(1 << nbits)
    sup = (idx[:, None] & idx[None, :]) == idx[None, :]
    pc = np.array([bin(x).count("1") for x in range(1 << nbits)])
    signs = (-1.0) ** pc[idx[:, None] & ~idx[None, :]]
    return (sup * signs).astype(np.float32)


def build_kernel():
    import concourse.bacc as bacc
    import concourse.tile as tile
    from concourse import mybir

    f32 = mybir.dt.float32
    nc = bacc.Bacc("TRN2", target_bir_lowering=False, debug=False, num_devices=NCORES)

    m1_in = nc.dram_tensor("m1", [P, F], f32, kind="ExternalInput")
    m2_in = nc.dram_tensor("m2", [P, F], f32, kind="ExternalInput")
    out_t = nc.dram_tensor("out", [P, F], f32, kind="ExternalOutput")

    WZ7_d = nc.inline_tensor(_zeta_mat(7), name="WZ7")
    WM7_d = nc.inline_tensor(_mobius_mat(7), name="WM7")
    WZ3_d = nc.inline_tensor(np.kron(_zeta_mat(3), np.eye(16, dtype=np.float32)), name="WZ3x")
    WM3_d = nc.inline_tensor(np.kron(_mobius_mat(3), np.eye(16, dtype=np.float32)), name="WM3x")

    with tile.TileContext(nc) as tc:
        with tc.tile_pool(name="sbuf", bufs=1) as pool, \
             tc.tile_pool(name="chunks", bufs=4) as cpool, \
             tc.tile_pool(name="psum", bufs=2, space="PSUM") as psum, \
             tc.tile_pool(name="dram", bufs=1, space="DRAM") as dram:

            wz7 = pool.tile([P, P], f32)
            wm7 = pool.tile([P, P], f32)
            wz3 = pool.tile([P, P], f32)
            wm3 = pool.tile([P, P], f32)
            nc.sync.dma_start(out=wz7[:], in_=WZ7_d[:])
            nc.sync.dma_start(out=wm7[:], in_=WM7_d[:])
            nc.sync.dma_start(out=wz3[:], in_=WZ3_d[:])
            nc.sync.dma_start(out=wm3[:], in_=WM3_d[:])

            A = pool.tile([P, F], f32)
            B = pool.tile([P, F], f32)
            nc.sync.dma_start(out=A[:], in_=m1_in[:])
            nc.scalar.dma_start(out=B[:], in_=m2_in[:])

            cc_in = [dram.tile([NCORES, 2, 16, MSL * BLK], f32, tag=f"cci{s}", name=f"cci{s}") for s in range(NSLAB)]
            cc_out = [dram.tile([NCORES, 2, 16, MSL * BLK], f32, tag=f"cco{s}", name=f"cco{s}") for s in range(NSLAB)]
            cc2_in = [dram.tile([NCORES, 16, MSL * BLK], f32, tag=f"c2i{s}", name=f"c2i{s}") for s in range(NSLAB)]
            cc2_out = [dram.tile([NCORES, 16, MSL * BLK], f32, tag=f"c2o{s}", name=f"c2o{s}") for s in range(NSLAB)]

            ADD = "add"

            def tt(eng, alu, lo, hi):
                f = getattr(nc.vector if eng == "v" else nc.gpsimd,
                            "tensor_add" if alu == ADD else "tensor_sub")
                f(lo, lo, hi)

            def bf_split(alu, lo, hi):
                # DVE only: GPSIMD shares SBUF ports with DVE, so
                # concurrent fp32 TT on both engines halves each rate.
                tt("v", alu, lo, hi)

            def s13(t, alu):
                # bit-13 butterfly: cols [0:8k] (+/-)= cols [8k:16k]
                tt("v", alu, t[:, 0:8192], t[:, 8192:16384])

            def s12a(t, alu):
                tt("v", alu, t[:, 0:4096], t[:, 4096:8192])

            def s12b(t, alu):
                tt("v", alu, t[:, 8192:12288], t[:, 12288:16384])

            def slab_stages(t, s, alu):
                # intra-slab stages j=0..11 on contiguous slab s
                sl = t[:, s * SLAB:(s + 1) * SLAB]
                for j in range(12):
                    w = sl.rearrange("p (a two b) -> p a two b", two=2, b=1 << j)
                    bf_split(alu, w[:, :, 0, :], w[:, :, 1, :])

            def mm_block(t, w, blk):
                ps = psum.tile([P, BLK], f32, tag="ps_mm")
                sl = t[:, blk * BLK:(blk + 1) * BLK]
                nc.tensor.matmul(ps[:], lhsT=w[:], rhs=sl, start=True, stop=True)
                nc.scalar.copy(sl, ps[:])

            def slab_ap(t, s, prange=None):
                if prange is None:
                    return t[:, s * SLAB:(s + 1) * SLAB]
                return t[prange[0]:prange[1], s * SLAB:(s + 1) * SLAB]

            def emit_phase3_back(s):
                # recv, h-conv, back-send, back A2A for slab s
                for c in range(NCORES):
                    nc.gpsimd.dma_start(out=slab_ap(A, s, (16 * c, 16 * (c + 1))), in_=cc_out[s][c, 0])
                    nc.gpsimd.dma_start(out=slab_ap(B, s, (16 * c, 16 * (c + 1))), in_=cc_out[s][c, 1])
                for m in range(MSL):
                    blk = s * MSL + m
                    sa = A[:, blk * BLK:(blk + 1) * BLK]
                    sb = B[:, blk * BLK:(blk + 1) * BLK]
                    psA = psum.tile([P, BLK], f32, tag="psA")
                    psB = psum.tile([P, BLK], f32, tag="psB")
                    nc.tensor.matmul(psA[:], lhsT=wz3[:], rhs=sa, start=True, stop=True)
                    nc.tensor.matmul(psB[:], lhsT=wz3[:], rhs=sb, start=True, stop=True)
                    qa = cpool.tile([P, BLK], f32, tag="qa")
                    nc.scalar.copy(qa[:], psA[:])
                    pr = cpool.tile([P, BLK], f32, tag="pr")
                    nc.vector.tensor_mul(pr[:], qa[:], psB[:])
                    psU = psum.tile([P, BLK], f32, tag="psU")
                    nc.tensor.matmul(psU[:], lhsT=wm3[:], rhs=pr[:], start=True, stop=True)
                    nc.scalar.copy(sa, psU[:])
                for c in range(NCORES):
                    nc.sync.dma_start(out=cc2_in[s][c], in_=slab_ap(A, s, (16 * c, 16 * (c + 1))))
                nc.gpsimd.collective_compute(
                    "AllToAll", mybir.AluOpType.bypass,
                    replica_groups=[list(range(NCORES))],
                    ins=[cc2_in[s][:].opt()], outs=[cc2_out[s][:].opt()],
                )

            # ---------- phase 1 + forward A2As, back path interleaved 2 behind ----------
            for t in (A, B):
                s13(t, ADD)
                s12a(t, ADD)
            for s in range(NSLAB):
                if s == 2:
                    # s12b (writes [8k:12k], feeds slabs 2,3) deferred out of
                    # the slab-0 critical path
                    s12b(A, ADD)
                    s12b(B, ADD)
                for t in (A, B):
                    slab_stages(t, s, ADD)
                    for m in range(MSL):
                        mm_block(t, wz7, s * MSL + m)
                for d in range(NCORES):
                    nc.sync.dma_start(out=cc_in[s][d, 0], in_=slab_ap(A, s, (16 * d, 16 * (d + 1))))
                    nc.sync.dma_start(out=cc_in[s][d, 1], in_=slab_ap(B, s, (16 * d, 16 * (d + 1))))
                nc.gpsimd.collective_compute(
                    "AllToAll", mybir.AluOpType.bypass,
                    replica_groups=[list(range(NCORES))],
                    ins=[cc_in[s][:].opt()], outs=[cc_out[s][:].opt()],
                )
                if s >= 2:
                    emit_phase3_back(s - 2)
            for s in (NSLAB - 2, NSLAB - 1):
                emit_phase3_back(s)

            # ---------- phase 5 per slab ----------
            for s in range(NSLAB):
                for d in range(NCORES):
                    nc.gpsimd.dma_start(out=slab_ap(B, s, (16 * d, 16 * (d + 1))), in_=cc2_out[s][d])
                for m in range(MSL):
                    mm_block(B, wm7, s * MSL + m)
                slab_stages(B, s, "sub")
                if s == 1:
                    # needs only slabs 0,1 — run while slabs 2,3 are in flight
                    s12a(B, "sub")

            # ---------- inverse cross-slab stages + streamed output ----------
            s12b(B, "sub")
            nc.sync.dma_start(out=out_t[:, 12288:16384], in_=B[:, 12288:16384])
            s13(B, "sub")
            nc.sync.dma_start(out=out_t[:, 8192:12288], in_=B[:, 8192:12288])
            nc.sync.dma_start(out=out_t[:, 0:8192], in_=B[:, 0:8192])

    nc.compile()
    return nc


_NC_CACHE = None


def kernel(m12: np.ndarray) -> np.ndarray:
    global _NC_CACHE
    from concourse.bass_utils import run_bass_kernel_spmd

    if _NC_CACHE is None:
        _NC_CACHE = build_kernel()
    nc = _NC_CACHE

    m12 = np.ascontiguousarray(np.asarray(m12, dtype=np.float32))
    Bsz, C, L = m12.shape
    S = L // NCORES
    in_maps = []
    for c in range(NCORES):
        in_maps.append({
            "m1": m12[0, 0, c * S:(c + 1) * S].reshape(P, F),
            "m2": m12[0, 1, c * S:(c + 1) * S].reshape(P, F),
        })
    try:
        res = run_bass_kernel_spmd(nc, in_maps, core_ids=list(range(NCORES)))
    except Exception:
        # transient NRT/device hiccups have been observed; retry once
        import time
        time.sleep(5)
        res = run_bass_kernel_spmd(nc, in_maps, core_ids=list(range(NCORES)))
    out = np.concatenate([res.results[c]["out"].reshape(-1) for c in range(NCORES)])
    return out.reshape(1, L, 1, 1)


if __name__ == "__main__":
    m12 = np.load("/root/problem/m12.npy")
    out = kernel(m12)
    exp = np.load("/root/problem/expected.npy")
    err = np.abs(out - exp).max()
    scale = np.abs(exp).max()
    print(f"absmax err {err:.4g} scale {scale:.4g} rel {err/scale:.3e}")

